# revision 15
# baseline (speedup 1.0000x reference)
"""Trainium2 Bass kernel for the CustomODELoss problem.

Full inputs:
    predicted_solution_batch [4096, 8192] f32
    target_solution_batch    [4096, 8192] f32
    c_input_batch            [4096]       f32
    x_eval_points            [8192]       f32   (uniform grid on [0, 1])

loss = mean((pred - target)^2)
     + mean((pred[r, idx_r] - 1)^2)
     + mean(((pred[r, idx_p] - pred[r, idx_m]) / ((idx_p - idx_m) * dx))^2)
where idx_r = argmin_j |x_j - c_r| (first index on ties).

Sharding: data-parallel over the batch dim, 512 rows per core on 8 cores.

Device-side work is the memory-bound part only: stream the pred/targ
slices once (sum of squared differences), plus one tiny 3-wide indirect
gather per row for the f(c) / f'(c) terms.  The per-row grid index
resolve (argmin over the uniform grid) runs on HOST numpy over the tiny
c / x_eval inputs with bit-identical f32 semantics to the reference
(same |x - c| values, same first-index tie-break); the device receives
precomputed gather offsets plus select/finite-difference WEIGHTS, so
f(c) = sum(w_fpc * window) and f'(c) = sum(w_fpp * window) are two
multiply+reduce pairs.

Streaming design notes (measured on HW traces):
  - pred rides the SP HWDGE ring (nc.sync), targ the Activation HWDGE
    ring (nc.scalar).  The 16 DMA engines strictly alternate between
    the two rings' packet queues, which hides the ~35ns/packet
    head-of-ring gap that left the engines ~14% idle on a single ring.
  - 2048-wide tiles (8 KiB per partition row per packet): DVE
    tensor_tensor at this width runs at ~214 G elem/s vs ~120 G at
    8192-wide/in-place, and small packets keep any ring-shared tiny
    transfer's round-robin latency low.
  - the tiny ints/wts loads and the pw gathers go through the GPSIMD
    SOFTWARE DGE ring: a [128, small] load is 128 separate packets
    that round-robin 1:1 with streaming packets per engine, so putting
    them on a HW ring ahead of the stream stalls that ring for
    ~(128/16)*packet_dur; on the SWDGE ring they only add ~3us of
    latency to the (off-critical-path) gather chain.
  - all compute runs on DVE (subtract, then square via
    scalar_tensor_tensor (d*1)*d with accum_out row-sums), so the
    Scalar engine's instruction stream is pure DMA triggers and the
    ~1.3us ACT_TABLE_LOAD never gates the targ ring.
  - last row block tapers (3x2048 + 2x1024) so the serial tail
    (last load -> subtract -> square -> reduce -> store) is ~2us.

The device emits per-partition partial sums [128, 3]; the host sums the
8 cores' partials in f64 and forms the three means.
"""

import numpy as np

import concourse.bacc as bacc
import concourse.bass as bass
import concourse.mybir as mybir
from concourse import tile
from concourse.bass_utils import run_bass_kernel_spmd

F32 = mybir.dt.float32
I32 = mybir.dt.int32
OP = mybir.AluOpType

B = 4096
N = 8192
NCORES = 8
BL = B // NCORES          # rows per core = 512
P = 128                   # SBUF partitions
RB = BL // P              # row groups per partition = 4
W = 3                     # gather window width
FT = 4096                 # streaming tile width

# (row_block, col_start, width) in STREAM order.  The taper must sit at
# the end of the stream order (not of any particular row block): compute
# (DVE subtract ~4.6us/pair + ACT square) is 2x faster than the
# ~9.6us/pair stream rate, so everything overlaps except the tiles that
# land last — tapering those (2048 -> 512) leaves only a ~0.7us subtract
# plus square after the final packet instead of a full row block's ~10us
# compute burst.
TILES = [
    (3, 0, 4096),
    (0, 0, 4096), (0, 4096, 4096),
    (1, 0, 4096), (1, 4096, 4096),
    (2, 0, 4096), (2, 4096, 4096),
    (3, 4096, 2048),
    (3, 6144, 1024),
    (3, 7168, 512),
    (3, 7680, 512),
]
NT = len(TILES)           # 11: 7 big + 4 taper


def build_nc(debug=False):
    # Bacc (not plain Bass): its compile pipeline runs
    # generate_event_semaphores, which splits multi-sem waits into separate
    # event instructions — TRN2 allows at most 1 embedded wait per
    # instruction, and walrus codegen rejects the unsplit form.
    nc = bacc.Bacc()

    # pred and targ are concatenated host-side into one [2*BL, N] tensor:
    # a single 3D-AP DMA then pulls a tile's pred AND targ rows together
    # (one trigger, one completion event, same packet count), halving the
    # per-load sync scaffolding and letting the two HWDGE rings alternate
    # whole tiles instead of splitting pred/targ
    cat = nc.dram_tensor("cat", [2 * BL, N], F32, kind="ExternalInput")
    catv = cat[:].rearrange("(a r) n -> r a n", a=2)   # [512, 2, N]
    # host-computed: flat gather offsets (row*N + clip(idx-1, 0, N-3)),
    # row r = p*RB + q
    ints = nc.dram_tensor("ints", [P, RB], I32, kind="ExternalInput")
    # host-computed weights: [:, 0:12] = f(c) one-hot select,
    # [:, 12:24] = f'(c) (+1/-1)/denom finite-difference weights,
    # both laid out [128, RB*W]
    wts = nc.dram_tensor("wts", [P, 2 * RB * W], F32, kind="ExternalInput")
    partials = nc.dram_tensor("partials", [P, 3], F32, kind="ExternalOutput")
    if debug:
        dbg = nc.dram_tensor("dbg", [P, 24], F32, kind="ExternalOutput")

    def view3(t):  # [128, 12] AP -> [128, 4, 3] AP
        return t.rearrange("p (q k) -> p q k", k=W)

    with tile.TileContext(nc) as tc:
        with (
            # one big-tile pool PER RING (2 slots x 32KB each): a shared
            # pool alternates slots across rings, leaving ~1.5 slots of
            # runway per ring — on contended-HBM runs a slot wait chained
            # to a data-gated subtract then starves a ring mid-stream for
            # >10us.  Per-ring pools give each ring 8MB of runway whose
            # slot releases depend only on that ring's own tiles.
            tc.tile_pool(name="apool", bufs=2) as apool,
            tc.tile_pool(name="bpool", bufs=2) as bpool,
            tc.tile_pool(name="dpool", bufs=2) as dpool,
            tc.tile_pool(name="pb", bufs=1) as pb,
        ):
            # tiny loads via SWDGE so neither HW ring stalls on them
            ints_t = pb.tile([P, RB], I32)
            nc.gpsimd.dma_start(ints_t[:], ints[:, :])
            wts_t = pb.tile([P, 2 * RB * W], F32)
            nc.gpsimd.dma_start(wts_t[:], wts[:, :])

            parts = pb.tile([P, NT], F32)
            po = pb.tile([P, 3], F32)

            ct = [None] * NT

            def load(k):
                rb, cs, w = TILES[k]
                rs = rb * P
                if w == FT:
                    pool, nm = (apool, "ca") if k % 2 == 0 else (bpool, "cb")
                    ct[k] = pool.tile([P, 2 * FT], F32, name=nm)
                else:
                    # taper tiles own dedicated one-shot buffers: their
                    # triggers carry NO pool-slot wait, so the rings blast
                    # through the small stream-end tiles back-to-back
                    # instead of starving on compute-gated slot releases
                    ct[k] = pb.tile([P, 2 * w], F32, name=f"ctl{k}")
                eng = nc.sync if k % 2 == 0 else nc.scalar
                eng.dma_start(
                    ct[k][:, :2 * w].rearrange("p (a j) -> p a j", a=2),
                    catv[rs:rs + P, :, cs:cs + w])

            def compute(k):
                _, _, w = TILES[k]
                dt = dpool.tile([P, FT], F32, name="dt")
                nc.vector.tensor_tensor(out=dt[:, :w], in0=ct[k][:, :w],
                                        in1=ct[k][:, w:2 * w], op=OP.subtract)
                # square + per-partition row-sum on the ACT engine so it
                # runs CONCURRENTLY with the next tile's DVE subtract — with
                # both on DVE the serial sub/sq chain of the taper tiles
                # adds ~8us of tail after the last load.  In place over dt
                # (the values are dead, only accum matters); dt has 2
                # rotating slots so sub_{k+1} waits sq_{k-1}, not sq_k —
                # bufs=1 here serializes DVE and ACT into a ping-pong that
                # paces the whole pipeline at sub+sq latency.
                nc.scalar.activation(
                    out=dt[:, :w], in_=dt[:, :w],
                    func=mybir.ActivationFunctionType.Square,
                    accum_out=parts[:, k:k + 1],
                )

            # ALL load triggers are emitted before any compute: a trigger
            # sitting after an ACTIVATE in the Scalar stream cannot enqueue
            # until that square retires, which starves the targ ring at
            # stream end.  Pool-slot embedded waits + the depth-4 HWDGE
            # ring throttle the stream correctly without engine-stream
            # ordering; the taper tiles' dedicated buffers carry no waits
            # at all.
            for k in range(NT):
                load(k)

            # gathers: 3-wide pred window per row via SWDGE; one offset
            # per partition per instruction (HW honors only one)
            pw = pb.tile([P, RB * W], F32)
            for q in range(RB):
                nc.gpsimd.indirect_dma_start(
                    out=pw[:, W * q:W * q + W], out_offset=None,
                    in_=cat[:, :],
                    in_offset=bass.IndirectOffsetOnAxis(
                        ap=ints_t[:, q:q + 1], axis=1),
                )

            for k in range(NT):
                compute(k)
                if k == 0:
                    # f(c) / f'(c): weighted 3-window sums; off the
                    # streaming critical path
                    sel = pb.tile([P, RB * W], F32)
                    nc.vector.tensor_tensor(out=sel[:], in0=wts_t[:, :RB * W],
                                            in1=pw[:], op=OP.mult)
                    fpc = pb.tile([P, RB], F32)
                    nc.vector.reduce_sum(out=fpc[:], in_=view3(sel[:]),
                                         axis=mybir.AxisListType.X)
                    fdw = pb.tile([P, RB * W], F32)
                    nc.vector.tensor_tensor(out=fdw[:], in0=wts_t[:, RB * W:],
                                            in1=pw[:], op=OP.mult)
                    fpp = pb.tile([P, RB], F32)
                    nc.vector.reduce_sum(out=fpp[:], in_=view3(fdw[:]),
                                         axis=mybir.AxisListType.X)
                    # term2: (f(c) - 1)^2; term3: f'(c)^2
                    fpm1 = pb.tile([P, RB], F32)
                    nc.vector.tensor_scalar(out=fpm1[:], in0=fpc[:],
                                            scalar1=-1.0, scalar2=None,
                                            op0=OP.add)
                    sq2 = pb.tile([P, RB], F32)
                    nc.vector.scalar_tensor_tensor(
                        out=sq2[:], in0=fpm1[:], scalar=1.0, in1=fpm1[:],
                        op0=OP.mult, op1=OP.mult, accum_out=po[:, 1:2])
                    sq3 = pb.tile([P, RB], F32)
                    nc.vector.scalar_tensor_tensor(
                        out=sq3[:], in0=fpp[:], scalar=1.0, in1=fpp[:],
                        op0=OP.mult, op1=OP.mult, accum_out=po[:, 2:3])
                    if debug:
                        dbt = pb.tile([P, 24], F32)
                        nc.vector.tensor_copy(out=dbt[:, 0:12], in_=pw[:])
                        nc.vector.tensor_copy(out=dbt[:, 12:16], in_=fpc[:])
                        nc.vector.tensor_copy(out=dbt[:, 16:20], in_=fpp[:])
                        offf = pb.tile([P, RB], F32)
                        nc.vector.tensor_copy(out=offf[:], in_=ints_t[:])
                        nc.vector.tensor_copy(out=dbt[:, 20:24], in_=offf[:])
                        nc.sync.dma_start(dbg[:, :], dbt[:])

            nc.vector.reduce_sum(out=po[:, 0:1], in_=parts[:],
                                 axis=mybir.AxisListType.X)
            nc.sync.dma_start(partials[:, :], po[:])

    return nc


_NC_CACHE = None


def _get_nc():
    global _NC_CACHE
    if _NC_CACHE is None:
        nc = build_nc()
        # Bacc runs its compile pipeline (register alloc, sync-wait
        # splitting) in finalize; the PJRT exec path requires it.
        nc.finalize()
        _NC_CACHE = nc
    return _NC_CACHE


def _host_index_prep(c, x):
    """Exact replication of the reference index math on the tiny inputs.

    idx = argmin_j |x_j - c_r| with numpy f32 ops — bit-identical values
    and the same first-index tie-break as jnp.argmin on CPU.
    Returns flat gather offsets into each core's [BL, N] pred slice and
    the f(c)/f'(c) window weights.
    """
    Bfull = c.shape[0]
    idx = np.empty(Bfull, dtype=np.int64)
    CH = 512
    for s in range(0, Bfull, CH):
        e = min(s + CH, Bfull)
        d = np.abs(x[None, :] - c[s:e, None])  # f32
        idx[s:e] = np.argmin(d, axis=1)
    dx = np.float32(x[1]) - np.float32(x[0])

    ip = np.minimum(idx + 1, N - 1)
    im = np.maximum(idx - 1, 0)
    s3 = np.clip(idx - 1, 0, N - W)           # window start
    p0 = (idx - s3).astype(np.int64)          # positions in window
    pm = (im - s3).astype(np.int64)
    pp = (ip - s3).astype(np.int64)
    denom = (ip - im).astype(np.float32) * dx
    rden = np.float32(1.0) / denom

    rows = np.arange(Bfull)
    wfpc = np.zeros((Bfull, W), dtype=np.float32)
    wfpc[rows, p0] = 1.0
    wfpp = np.zeros((Bfull, W), dtype=np.float32)
    # += not =: pm and pp never collide (pm < pp always since ip > im),
    # but keep the accumulate form cheap and safe
    np.add.at(wfpp, (rows, pp), rden)
    np.add.at(wfpp, (rows, pm), -rden)

    row_in_core = np.arange(Bfull) % BL
    offs = (row_in_core * N + s3).astype(np.int32)
    return offs, wfpc, wfpp


def make_in_maps(predicted_solution_batch, target_solution_batch,
                 c_input_batch, x_eval_points):
    pred = np.ascontiguousarray(predicted_solution_batch, dtype=np.float32)
    targ = np.ascontiguousarray(target_solution_batch, dtype=np.float32)
    c = np.ascontiguousarray(c_input_batch, dtype=np.float32)
    x = np.ascontiguousarray(x_eval_points, dtype=np.float32)
    offs, wfpc, wfpp = _host_index_prep(c, x)

    in_maps = []
    for i in range(NCORES):
        sl = slice(i * BL, (i + 1) * BL)
        # row r in core = p*RB + q  ->  [P, RB] / [P, RB*W] layouts
        wf1 = wfpc[sl].reshape(P, RB * W)
        wf2 = wfpp[sl].reshape(P, RB * W)
        in_maps.append({
            "cat": np.ascontiguousarray(
                np.concatenate([pred[sl], targ[sl]], axis=0)),
            "ints": offs[sl].reshape(P, RB),
            "wts": np.ascontiguousarray(np.concatenate([wf1, wf2], axis=1)),
        })
    return in_maps


def reduce_partials(results):
    s = np.zeros(3, dtype=np.float64)
    for r in results:
        s += r["partials"].astype(np.float64).sum(axis=0)
    loss = s[0] / (B * N) + s[1] / B + s[2] / B
    return np.float32(loss)


def kernel(predicted_solution_batch, target_solution_batch,
           c_input_batch, x_eval_points):
    nc = _get_nc()
    in_maps = make_in_maps(predicted_solution_batch, target_solution_batch,
                           c_input_batch, x_eval_points)
    res = run_bass_kernel_spmd(nc, in_maps, core_ids=list(range(NCORES)))
    return reduce_partials(res.results)


# revision 16
# speedup vs baseline: 1.0612x; 1.0612x over previous
"""Trainium2 Bass kernel for the CustomODELoss problem.

Full inputs:
    predicted_solution_batch [4096, 8192] f32
    target_solution_batch    [4096, 8192] f32
    c_input_batch            [4096]       f32
    x_eval_points            [8192]       f32   (uniform grid on [0, 1])

loss = mean((pred - target)^2)
     + mean((pred[r, idx_r] - 1)^2)
     + mean(((pred[r, idx_p] - pred[r, idx_m]) / ((idx_p - idx_m) * dx))^2)
where idx_r = argmin_j |x_j - c_r| (first index on ties).

Sharding: data-parallel over the batch dim, 512 rows per core on 8 cores.

Device-side work is the memory-bound part: stream the pred/targ slices
once (sum of squared differences), plus one tiny 3-wide indirect gather
per row for the f(c) / f'(c) terms.  The per-row grid index resolve
(argmin over the uniform grid) runs on HOST numpy over the tiny
c / x_eval inputs with bit-identical f32 semantics to the reference
(same |x - c| values, same first-index tie-break); the device receives
precomputed gather offsets plus select/finite-difference WEIGHTS, so
f(c) = sum(w_fpc * window) and f'(c) = sum(w_fpp * window) are two
multiply+reduce pairs feeding two square-accumulates.

Streaming design (each point validated against HW traces):
  - pred rides the SP HWDGE ring (nc.sync), targ the Activation ring
    (nc.scalar).  The 16 DMA engines round-robin both rings' packet
    queues, hiding the ~35ns/packet head-of-ring gap that leaves ~14%
    idle on a single ring (measured 99% engine duty / ~416 GB/s on
    uncontended runs).
  - 4096-wide tiles: 16 KiB contiguous DRAM per packet.  pred and targ
    stay SEPARATE tiles — a merged [pred|targ] tile halves DVE/ACT
    throughput via SBUF bank conflicts between the two read streams.
  - ALL load triggers are emitted before any compute: a trigger behind
    an ACTIVATE in the Scalar stream cannot enqueue until that square
    retires, starving the targ ring at stream end.  Pool-slot embedded
    waits plus the depth-4 HWDGE ring throttle the stream correctly.
  - pools are per ring (pred pool / targ pool, 3 slots each), so a slot
    wait only chains to that ring's own subtracts; slots release at the
    subtract (squares write a separate scratch), keeping triggers a
    full square-latency ahead.
  - the last 4096 columns of row block 3 taper 2048/1024/512/512 and
    stream LAST, in dedicated one-shot buffers (no pool waits): compute
    (sub ~2.3us + ACT square in parallel) is 2x faster than the
    ~9.6us/pair stream rate, so after the final 512-wide packet only
    ~1.5us of work remains.
  - the tiny ints/wts loads and the pw gathers use the GPSIMD software
    DGE: a [128, small] load is 128 separate packets that round-robin
    1:1 with streaming packets per engine, so on a HW ring ahead of the
    stream they would stall that ring ~(128/16)*packet_dur.
  - subtract on DVE, square+row-sum accumulate on ACT (concurrent
    engines), one [128, 3] output store at the end.

The device emits per-partition partial sums [128, 3]; the host sums the
8 cores' partials in f64 and forms the three means.
"""

import numpy as np

import concourse.bacc as bacc
import concourse.bass as bass
import concourse.mybir as mybir
from concourse import tile
from concourse.bass_utils import run_bass_kernel_spmd

F32 = mybir.dt.float32
I32 = mybir.dt.int32
OP = mybir.AluOpType

B = 4096
N = 8192
NCORES = 8
BL = B // NCORES          # rows per core = 512
P = 128                   # SBUF partitions
RB = BL // P              # row groups per partition = 4
W = 3                     # gather window width
FT = 4096                 # streaming tile width

# (row_block, col_start, width) in STREAM order; taper last
TILES = [
    (3, 0, 4096),
    (0, 0, 4096), (0, 4096, 4096),
    (1, 0, 4096), (1, 4096, 4096),
    (2, 0, 4096), (2, 4096, 4096),
    (3, 4096, 2048),
    (3, 6144, 1024),
    (3, 7168, 512),
    (3, 7680, 512),
]
NT = len(TILES)           # 11: 7 big + 4 taper


def build_nc(debug=False):
    # Bacc (not plain Bass): its compile pipeline runs
    # generate_event_semaphores, which splits multi-sem waits into separate
    # event instructions — TRN2 allows at most 1 embedded wait per
    # instruction, and walrus codegen rejects the unsplit form.
    nc = bacc.Bacc()

    pred = nc.dram_tensor("pred", [BL, N], F32, kind="ExternalInput")
    targ = nc.dram_tensor("targ", [BL, N], F32, kind="ExternalInput")
    # host-computed: flat gather offsets (row*N + clip(idx-1, 0, N-3)),
    # row r = p*RB + q
    ints = nc.dram_tensor("ints", [P, RB], I32, kind="ExternalInput")
    # host-computed weights: [:, 0:12] = f(c) one-hot select,
    # [:, 12:24] = f'(c) (+1/-1)/denom finite-difference weights,
    # both laid out [128, RB*W]
    wts = nc.dram_tensor("wts", [P, 2 * RB * W], F32, kind="ExternalInput")
    partials = nc.dram_tensor("partials", [P, 3], F32, kind="ExternalOutput")
    if debug:
        dbg = nc.dram_tensor("dbg", [P, 24], F32, kind="ExternalOutput")

    def view3(t):  # [128, 12] AP -> [128, 4, 3] AP
        return t.rearrange("p (q k) -> p q k", k=W)

    with tile.TileContext(nc) as tc:
        with (
            tc.tile_pool(name="ppool", bufs=3) as ppool,
            tc.tile_pool(name="tpool", bufs=3) as tpool,
            tc.tile_pool(name="dpool", bufs=2) as dpool,
            tc.tile_pool(name="pb", bufs=1) as pb,
        ):
            # tiny loads via SWDGE so neither HW ring stalls on them
            ints_t = pb.tile([P, RB], I32)
            nc.gpsimd.dma_start(ints_t[:], ints[:, :])
            wts_t = pb.tile([P, 2 * RB * W], F32)
            nc.gpsimd.dma_start(wts_t[:], wts[:, :])

            parts = pb.tile([P, NT], F32)
            po = pb.tile([P, 3], F32)

            pt = [None] * NT
            tt = [None] * NT

            def load(k):
                rb, cs, w = TILES[k]
                rs = rb * P
                if w == FT:
                    pt[k] = ppool.tile([P, FT], F32, name="pt")
                    tt[k] = tpool.tile([P, FT], F32, name="tt")
                else:
                    pt[k] = pb.tile([P, w], F32, name=f"ptl{k}")
                    tt[k] = pb.tile([P, w], F32, name=f"ttl{k}")
                nc.sync.dma_start(pt[k][:, :w], pred[rs:rs + P, cs:cs + w])
                nc.scalar.dma_start(tt[k][:, :w], targ[rs:rs + P, cs:cs + w])

            def compute(k):
                _, _, w = TILES[k]
                dt = dpool.tile([P, FT], F32, name="dt")
                nc.vector.tensor_tensor(out=dt[:, :w], in0=pt[k][:, :w],
                                        in1=tt[k][:, :w], op=OP.subtract)
                # ACT square runs concurrently with the next DVE subtract;
                # separate scratch out (in-place at 4096-wide runs 2x slow)
                st = dpool.tile([P, FT], F32, name="st")
                nc.scalar.activation(
                    out=st[:, :w], in_=dt[:, :w],
                    func=mybir.ActivationFunctionType.Square,
                    accum_out=parts[:, k:k + 1],
                )

            for k in range(NT):
                load(k)

            # gathers: 3-wide pred window per row via SWDGE; one offset
            # per partition per instruction (HW honors only one)
            pw = pb.tile([P, RB * W], F32)
            for q in range(RB):
                nc.gpsimd.indirect_dma_start(
                    out=pw[:, W * q:W * q + W], out_offset=None,
                    in_=pred[:, :],
                    in_offset=bass.IndirectOffsetOnAxis(
                        ap=ints_t[:, q:q + 1], axis=1),
                )

            for k in range(NT):
                compute(k)
                if k == 0:
                    # f(c) / f'(c): weighted 3-window sums; off the
                    # streaming critical path
                    sel = pb.tile([P, RB * W], F32)
                    nc.vector.tensor_tensor(out=sel[:], in0=wts_t[:, :RB * W],
                                            in1=pw[:], op=OP.mult)
                    fpc = pb.tile([P, RB], F32)
                    nc.vector.reduce_sum(out=fpc[:], in_=view3(sel[:]),
                                         axis=mybir.AxisListType.X)
                    fdw = pb.tile([P, RB * W], F32)
                    nc.vector.tensor_tensor(out=fdw[:], in0=wts_t[:, RB * W:],
                                            in1=pw[:], op=OP.mult)
                    fpp = pb.tile([P, RB], F32)
                    nc.vector.reduce_sum(out=fpp[:], in_=view3(fdw[:]),
                                         axis=mybir.AxisListType.X)
                    # term2: (f(c) - 1)^2; term3: f'(c)^2 — on DVE so the
                    # Scalar stream stays triggers + stream squares only
                    fpm1 = pb.tile([P, RB], F32)
                    nc.vector.tensor_scalar(out=fpm1[:], in0=fpc[:],
                                            scalar1=-1.0, scalar2=None,
                                            op0=OP.add)
                    sq2 = pb.tile([P, RB], F32)
                    nc.vector.scalar_tensor_tensor(
                        out=sq2[:], in0=fpm1[:], scalar=1.0, in1=fpm1[:],
                        op0=OP.mult, op1=OP.mult, accum_out=po[:, 1:2])
                    sq3 = pb.tile([P, RB], F32)
                    nc.vector.scalar_tensor_tensor(
                        out=sq3[:], in0=fpp[:], scalar=1.0, in1=fpp[:],
                        op0=OP.mult, op1=OP.mult, accum_out=po[:, 2:3])
                    if debug:
                        dbt = pb.tile([P, 24], F32)
                        nc.vector.tensor_copy(out=dbt[:, 0:12], in_=pw[:])
                        nc.vector.tensor_copy(out=dbt[:, 12:16], in_=fpc[:])
                        nc.vector.tensor_copy(out=dbt[:, 16:20], in_=fpp[:])
                        offf = pb.tile([P, RB], F32)
                        nc.vector.tensor_copy(out=offf[:], in_=ints_t[:])
                        nc.vector.tensor_copy(out=dbt[:, 20:24], in_=offf[:])
                        nc.sync.dma_start(dbg[:, :], dbt[:])

            nc.vector.reduce_sum(out=po[:, 0:1], in_=parts[:],
                                 axis=mybir.AxisListType.X)
            nc.sync.dma_start(partials[:, :], po[:])

    return nc


_NC_CACHE = None


def _get_nc():
    global _NC_CACHE
    if _NC_CACHE is None:
        nc = build_nc()
        # Bacc runs its compile pipeline (register alloc, sync-wait
        # splitting) in finalize; the PJRT exec path requires it.
        nc.finalize()
        _NC_CACHE = nc
    return _NC_CACHE


def _host_index_prep(c, x):
    """Exact replication of the reference index math on the tiny inputs.

    idx = argmin_j |x_j - c_r| with numpy f32 ops — bit-identical values
    and the same first-index tie-break as jnp.argmin on CPU.
    Returns flat gather offsets into each core's [BL, N] pred slice and
    the f(c)/f'(c) window weights.
    """
    Bfull = c.shape[0]
    idx = np.empty(Bfull, dtype=np.int64)
    CH = 512
    for s in range(0, Bfull, CH):
        e = min(s + CH, Bfull)
        d = np.abs(x[None, :] - c[s:e, None])  # f32
        idx[s:e] = np.argmin(d, axis=1)
    dx = np.float32(x[1]) - np.float32(x[0])

    ip = np.minimum(idx + 1, N - 1)
    im = np.maximum(idx - 1, 0)
    s3 = np.clip(idx - 1, 0, N - W)           # window start
    p0 = (idx - s3).astype(np.int64)          # positions in window
    pm = (im - s3).astype(np.int64)
    pp = (ip - s3).astype(np.int64)
    denom = (ip - im).astype(np.float32) * dx
    rden = np.float32(1.0) / denom

    rows = np.arange(Bfull)
    wfpc = np.zeros((Bfull, W), dtype=np.float32)
    wfpc[rows, p0] = 1.0
    wfpp = np.zeros((Bfull, W), dtype=np.float32)
    # += not =: pm and pp never collide (ip > im always), but keep the
    # accumulate form cheap and safe
    np.add.at(wfpp, (rows, pp), rden)
    np.add.at(wfpp, (rows, pm), -rden)

    row_in_core = np.arange(Bfull) % BL
    offs = (row_in_core * N + s3).astype(np.int32)
    return offs, wfpc, wfpp


def make_in_maps(predicted_solution_batch, target_solution_batch,
                 c_input_batch, x_eval_points):
    pred = np.ascontiguousarray(predicted_solution_batch, dtype=np.float32)
    targ = np.ascontiguousarray(target_solution_batch, dtype=np.float32)
    c = np.ascontiguousarray(c_input_batch, dtype=np.float32)
    x = np.ascontiguousarray(x_eval_points, dtype=np.float32)
    offs, wfpc, wfpp = _host_index_prep(c, x)

    in_maps = []
    for i in range(NCORES):
        sl = slice(i * BL, (i + 1) * BL)
        # row r in core = p*RB + q  ->  [P, RB] / [P, RB*W] layouts
        wf1 = wfpc[sl].reshape(P, RB * W)
        wf2 = wfpp[sl].reshape(P, RB * W)
        in_maps.append({
            "pred": pred[sl],
            "targ": targ[sl],
            "ints": offs[sl].reshape(P, RB),
            "wts": np.ascontiguousarray(np.concatenate([wf1, wf2], axis=1)),
        })
    return in_maps


def reduce_partials(results):
    s = np.zeros(3, dtype=np.float64)
    for r in results:
        s += r["partials"].astype(np.float64).sum(axis=0)
    loss = s[0] / (B * N) + s[1] / B + s[2] / B
    return np.float32(loss)


def kernel(predicted_solution_batch, target_solution_batch,
           c_input_batch, x_eval_points):
    nc = _get_nc()
    in_maps = make_in_maps(predicted_solution_batch, target_solution_batch,
                           c_input_batch, x_eval_points)
    res = run_bass_kernel_spmd(nc, in_maps, core_ids=list(range(NCORES)))
    return reduce_partials(res.results)


# revision 17
# speedup vs baseline: 1.1115x; 1.0475x over previous
"""Trainium2 Bass kernel for the CustomODELoss problem.

Full inputs:
    predicted_solution_batch [4096, 8192] f32
    target_solution_batch    [4096, 8192] f32
    c_input_batch            [4096]       f32
    x_eval_points            [8192]       f32   (uniform grid on [0, 1])

loss = mean((pred - target)^2)
     + mean((pred[r, idx_r] - 1)^2)
     + mean(((pred[r, idx_p] - pred[r, idx_m]) / ((idx_p - idx_m) * dx))^2)
where idx_r = argmin_j |x_j - c_r| (first index on ties).

Sharding: data-parallel over the batch dim, 512 rows per core on 8 cores.

Device-side work is the memory-bound part: stream the pred/targ slices
once (sum of squared differences), plus one tiny 3-wide indirect gather
per row for the f(c) / f'(c) terms.  The per-row grid index resolve
(argmin over the uniform grid) runs on HOST numpy over the tiny
c / x_eval inputs with bit-identical f32 semantics to the reference
(same |x - c| values, same first-index tie-break); the device receives
precomputed gather offsets plus select/finite-difference WEIGHTS, so
f(c) = sum(w_fpc * window) and f'(c) = sum(w_fpp * window) are two
multiply+reduce pairs feeding two square-accumulates.

Streaming design (each point validated against HW traces):
  - pred rides the SP HWDGE ring (nc.sync), targ the Activation ring
    (nc.scalar).  The 16 DMA engines round-robin both rings' packet
    queues, hiding the ~35ns/packet head-of-ring gap that leaves ~14%
    idle on a single ring (measured 99% engine duty / ~416 GB/s on
    uncontended runs).
  - 4096-wide tiles: 16 KiB contiguous DRAM per packet.  pred and targ
    stay SEPARATE tiles — a merged [pred|targ] tile halves DVE/ACT
    throughput via SBUF bank conflicts between the two read streams.
  - ALL load triggers are emitted before any compute: a trigger behind
    an ACTIVATE in the Scalar stream cannot enqueue until that square
    retires, starving the targ ring at stream end.  Pool-slot embedded
    waits plus the depth-4 HWDGE ring throttle the stream correctly.
  - pools are per ring (pred pool / targ pool, 3 slots each), so a slot
    wait only chains to that ring's own subtracts; slots release at the
    subtract (squares write a separate scratch), keeping triggers a
    full square-latency ahead.
  - the last 4096 columns of row block 3 taper 2048/1024/512/512 and
    stream LAST, in dedicated one-shot buffers (no pool waits): compute
    (sub ~2.3us + ACT square in parallel) is 2x faster than the
    ~9.6us/pair stream rate, so after the final 512-wide packet only
    ~1.5us of work remains.
  - the tiny ints/wts loads and the pw gathers use the GPSIMD software
    DGE: a [128, small] load is 128 separate packets that round-robin
    1:1 with streaming packets per engine, so on a HW ring ahead of the
    stream they would stall that ring ~(128/16)*packet_dur.
  - subtract on DVE, square+row-sum accumulate on ACT (concurrent
    engines), one [128, 3] output store at the end.

The device emits per-partition partial sums [128, 3]; the host sums the
8 cores' partials in f64 and forms the three means.
"""

import numpy as np

import concourse.bacc as bacc
import concourse.bass as bass
import concourse.mybir as mybir
from concourse import tile
from concourse.bass_utils import run_bass_kernel_spmd

F32 = mybir.dt.float32
I32 = mybir.dt.int32
OP = mybir.AluOpType

B = 4096
N = 8192
NCORES = 8
BL = B // NCORES          # rows per core = 512
P = 128                   # SBUF partitions
RB = BL // P              # row groups per partition = 4
W = 3                     # gather window width
FT = 4096                 # streaming tile width

# (row_block, col_start, width) in STREAM order.  The taper sits at the
# end of the STREAM order (cross-block): compute is 2x faster than the
# stream, so everything overlaps except the last-landing tiles.
TILES = [
    (3, 0, 4096),
    (0, 0, 4096), (0, 4096, 4096),
    (1, 0, 4096), (1, 4096, 4096),
    (2, 0, 4096),
    (3, 4096, 2048), (2, 4096, 2048),
    (3, 6144, 1024), (2, 6144, 1024),
    (3, 7168, 512), (2, 7168, 512),
    (3, 7680, 512), (2, 7680, 512),
]
NT = len(TILES)           # 14: 6 big + 8 taper
NFILL = 3


def build_nc(debug=False):
    # Bacc (not plain Bass): its compile pipeline runs
    # generate_event_semaphores, which splits multi-sem waits into separate
    # event instructions — TRN2 allows at most 1 embedded wait per
    # instruction, and walrus codegen rejects the unsplit form.
    nc = bacc.Bacc()

    pred = nc.dram_tensor("pred", [BL, N], F32, kind="ExternalInput")
    targ = nc.dram_tensor("targ", [BL, N], F32, kind="ExternalInput")
    # host-computed: flat gather offsets (row*N + clip(idx-1, 0, N-3)),
    # row r = p*RB + q
    ints = nc.dram_tensor("ints", [P, RB], I32, kind="ExternalInput")
    # host-computed weights: [:, 0:12] = f(c) one-hot select,
    # [:, 12:24] = f'(c) (+1/-1)/denom finite-difference weights,
    # both laid out [128, RB*W]
    wts = nc.dram_tensor("wts", [P, 2 * RB * W], F32, kind="ExternalInput")
    partials = nc.dram_tensor("partials", [P, 3], F32, kind="ExternalOutput")
    if debug:
        dbg = nc.dram_tensor("dbg", [P, 24], F32, kind="ExternalOutput")

    def view3(t):  # [128, 12] AP -> [128, 4, 3] AP
        return t.rearrange("p (q k) -> p q k", k=W)

    with tile.TileContext(nc) as tc:
        with (
            tc.tile_pool(name="ppool", bufs=3) as ppool,
            tc.tile_pool(name="tpool", bufs=3) as tpool,
            tc.tile_pool(name="dpool", bufs=2) as dpool,
            tc.tile_pool(name="pb", bufs=1) as pb,
        ):
            # tiny loads via SWDGE so neither HW ring stalls on them
            ints_t = pb.tile([P, RB], I32)
            nc.gpsimd.dma_start(ints_t[:], ints[:, :])
            wts_t = pb.tile([P, 2 * RB * W], F32)
            nc.gpsimd.dma_start(wts_t[:], wts[:, :])

            parts = pb.tile([P, NT], F32)
            po = pb.tile([P, 3], F32)

            pt = [None] * NT
            tt = [None] * NT

            def load(k):
                rb, cs, w = TILES[k]
                rs = rb * P
                if w == FT:
                    pt[k] = ppool.tile([P, FT], F32, name="pt")
                    tt[k] = tpool.tile([P, FT], F32, name="tt")
                else:
                    pt[k] = pb.tile([P, w], F32, name=f"ptl{k}")
                    tt[k] = pb.tile([P, w], F32, name=f"ttl{k}")
                nc.sync.dma_start(pt[k][:, :w], pred[rs:rs + P, cs:cs + w])
                nc.scalar.dma_start(tt[k][:, :w], targ[rs:rs + P, cs:cs + w])

            def compute(k):
                _, _, w = TILES[k]
                dt = dpool.tile([P, FT], F32, name="dt")
                nc.vector.tensor_tensor(out=dt[:, :w], in0=pt[k][:, :w],
                                        in1=tt[k][:, :w], op=OP.subtract)
                # ACT square runs concurrently with the next DVE subtract;
                # in place over dt (values dead, only accum matters); dt has
                # 2 rotating slots so sub_{k+1} waits sq_{k-1}, not sq_k
                nc.scalar.activation(
                    out=dt[:, :w], in_=dt[:, :w],
                    func=mybir.ActivationFunctionType.Square,
                    accum_out=parts[:, k:k + 1],
                )

            for k in range(NFILL):
                load(k)

            # gathers: 3-wide pred window per row via SWDGE; one offset
            # per partition per instruction (HW honors only one)
            pw = pb.tile([P, RB * W], F32)
            for q in range(RB):
                nc.gpsimd.indirect_dma_start(
                    out=pw[:, W * q:W * q + W], out_offset=None,
                    in_=pred[:, :],
                    in_offset=bass.IndirectOffsetOnAxis(
                        ap=ints_t[:, q:q + 1], axis=1),
                )

            for k in range(NT):
                compute(k)
                if k == 0:
                    # f(c) / f'(c): weighted 3-window sums; off the
                    # streaming critical path
                    sel = pb.tile([P, RB * W], F32)
                    nc.vector.tensor_tensor(out=sel[:], in0=wts_t[:, :RB * W],
                                            in1=pw[:], op=OP.mult)
                    fpc = pb.tile([P, RB], F32)
                    nc.vector.reduce_sum(out=fpc[:], in_=view3(sel[:]),
                                         axis=mybir.AxisListType.X)
                    fdw = pb.tile([P, RB * W], F32)
                    nc.vector.tensor_tensor(out=fdw[:], in0=wts_t[:, RB * W:],
                                            in1=pw[:], op=OP.mult)
                    fpp = pb.tile([P, RB], F32)
                    nc.vector.reduce_sum(out=fpp[:], in_=view3(fdw[:]),
                                         axis=mybir.AxisListType.X)
                    # term2: (f(c) - 1)^2; term3: f'(c)^2 — on DVE so the
                    # Scalar stream stays triggers + stream squares only
                    fpm1 = pb.tile([P, RB], F32)
                    nc.vector.tensor_scalar(out=fpm1[:], in0=fpc[:],
                                            scalar1=-1.0, scalar2=None,
                                            op0=OP.add)
                    sq2 = pb.tile([P, RB], F32)
                    nc.vector.scalar_tensor_tensor(
                        out=sq2[:], in0=fpm1[:], scalar=1.0, in1=fpm1[:],
                        op0=OP.mult, op1=OP.mult, accum_out=po[:, 1:2])
                    sq3 = pb.tile([P, RB], F32)
                    nc.vector.scalar_tensor_tensor(
                        out=sq3[:], in0=fpp[:], scalar=1.0, in1=fpp[:],
                        op0=OP.mult, op1=OP.mult, accum_out=po[:, 2:3])
                    if debug:
                        dbt = pb.tile([P, 24], F32)
                        nc.vector.tensor_copy(out=dbt[:, 0:12], in_=pw[:])
                        nc.vector.tensor_copy(out=dbt[:, 12:16], in_=fpc[:])
                        nc.vector.tensor_copy(out=dbt[:, 16:20], in_=fpp[:])
                        offf = pb.tile([P, RB], F32)
                        nc.vector.tensor_copy(out=offf[:], in_=ints_t[:])
                        nc.vector.tensor_copy(out=dbt[:, 20:24], in_=offf[:])
                        nc.sync.dma_start(dbg[:, :], dbt[:])
                if k + NFILL < NT:
                    load(k + NFILL)

            nc.vector.reduce_sum(out=po[:, 0:1], in_=parts[:],
                                 axis=mybir.AxisListType.X)
            nc.sync.dma_start(partials[:, :], po[:])

    return nc


_NC_CACHE = None


def _get_nc():
    global _NC_CACHE
    if _NC_CACHE is None:
        nc = build_nc()
        # Bacc runs its compile pipeline (register alloc, sync-wait
        # splitting) in finalize; the PJRT exec path requires it.
        nc.finalize()
        _NC_CACHE = nc
    return _NC_CACHE


def _host_index_prep(c, x):
    """Exact replication of the reference index math on the tiny inputs.

    idx = argmin_j |x_j - c_r| with numpy f32 ops — bit-identical values
    and the same first-index tie-break as jnp.argmin on CPU.
    Returns flat gather offsets into each core's [BL, N] pred slice and
    the f(c)/f'(c) window weights.
    """
    Bfull = c.shape[0]
    idx = np.empty(Bfull, dtype=np.int64)
    CH = 512
    for s in range(0, Bfull, CH):
        e = min(s + CH, Bfull)
        d = np.abs(x[None, :] - c[s:e, None])  # f32
        idx[s:e] = np.argmin(d, axis=1)
    dx = np.float32(x[1]) - np.float32(x[0])

    ip = np.minimum(idx + 1, N - 1)
    im = np.maximum(idx - 1, 0)
    s3 = np.clip(idx - 1, 0, N - W)           # window start
    p0 = (idx - s3).astype(np.int64)          # positions in window
    pm = (im - s3).astype(np.int64)
    pp = (ip - s3).astype(np.int64)
    denom = (ip - im).astype(np.float32) * dx
    rden = np.float32(1.0) / denom

    rows = np.arange(Bfull)
    wfpc = np.zeros((Bfull, W), dtype=np.float32)
    wfpc[rows, p0] = 1.0
    wfpp = np.zeros((Bfull, W), dtype=np.float32)
    # += not =: pm and pp never collide (ip > im always), but keep the
    # accumulate form cheap and safe
    np.add.at(wfpp, (rows, pp), rden)
    np.add.at(wfpp, (rows, pm), -rden)

    row_in_core = np.arange(Bfull) % BL
    offs = (row_in_core * N + s3).astype(np.int32)
    return offs, wfpc, wfpp


def make_in_maps(predicted_solution_batch, target_solution_batch,
                 c_input_batch, x_eval_points):
    pred = np.ascontiguousarray(predicted_solution_batch, dtype=np.float32)
    targ = np.ascontiguousarray(target_solution_batch, dtype=np.float32)
    c = np.ascontiguousarray(c_input_batch, dtype=np.float32)
    x = np.ascontiguousarray(x_eval_points, dtype=np.float32)
    offs, wfpc, wfpp = _host_index_prep(c, x)

    in_maps = []
    for i in range(NCORES):
        sl = slice(i * BL, (i + 1) * BL)
        # row r in core = p*RB + q  ->  [P, RB] / [P, RB*W] layouts
        wf1 = wfpc[sl].reshape(P, RB * W)
        wf2 = wfpp[sl].reshape(P, RB * W)
        in_maps.append({
            "pred": pred[sl],
            "targ": targ[sl],
            "ints": offs[sl].reshape(P, RB),
            "wts": np.ascontiguousarray(np.concatenate([wf1, wf2], axis=1)),
        })
    return in_maps


def reduce_partials(results):
    s = np.zeros(3, dtype=np.float64)
    for r in results:
        s += r["partials"].astype(np.float64).sum(axis=0)
    loss = s[0] / (B * N) + s[1] / B + s[2] / B
    return np.float32(loss)


def kernel(predicted_solution_batch, target_solution_batch,
           c_input_batch, x_eval_points):
    nc = _get_nc()
    in_maps = make_in_maps(predicted_solution_batch, target_solution_batch,
                           c_input_batch, x_eval_points)
    res = run_bass_kernel_spmd(nc, in_maps, core_ids=list(range(NCORES)))
    return reduce_partials(res.results)


# revision 18
# speedup vs baseline: 1.1436x; 1.0289x over previous
"""Trainium2 Bass kernel for the CustomODELoss problem.

Full inputs:
    predicted_solution_batch [4096, 8192] f32
    target_solution_batch    [4096, 8192] f32
    c_input_batch            [4096]       f32
    x_eval_points            [8192]       f32   (uniform grid on [0, 1])

loss = mean((pred - target)^2)
     + mean((pred[r, idx_r] - 1)^2)
     + mean(((pred[r, idx_p] - pred[r, idx_m]) / ((idx_p - idx_m) * dx))^2)
where idx_r = argmin_j |x_j - c_r| (first index on ties).

Sharding: data-parallel over the batch dim, 512 rows per core on 8 cores.

Device-side work is the memory-bound part: stream the pred/targ slices
once (sum of squared differences), plus one tiny 3-wide indirect gather
per row for the f(c) / f'(c) terms.  The per-row grid index resolve
(argmin over the uniform grid) runs on HOST numpy over the tiny
c / x_eval inputs with bit-identical f32 semantics to the reference
(same |x - c| values, same first-index tie-break); the device receives
precomputed gather offsets plus select/finite-difference WEIGHTS, so
f(c) = sum(w_fpc * window) and f'(c) = sum(w_fpp * window) are two
multiply+reduce pairs feeding two square-accumulates.

Streaming design (each point validated against HW traces):
  - pred rides the SP HWDGE ring (nc.sync), targ the Activation ring
    (nc.scalar).  The 16 DMA engines round-robin both rings' packet
    queues, hiding the ~35ns/packet head-of-ring gap that leaves ~14%
    idle on a single ring (measured 99% engine duty / ~416 GB/s on
    uncontended runs).
  - 4096-wide tiles: 16 KiB contiguous DRAM per packet.  pred and targ
    stay SEPARATE tiles — a merged [pred|targ] tile halves DVE/ACT
    throughput via SBUF bank conflicts between the two read streams.
  - ALL load triggers are emitted before any compute: a trigger behind
    an ACTIVATE in the Scalar stream cannot enqueue until that square
    retires, starving the targ ring at stream end.  Pool-slot embedded
    waits plus the depth-4 HWDGE ring throttle the stream correctly.
  - pools are per ring (pred pool / targ pool, 3 slots each), so a slot
    wait only chains to that ring's own subtracts; slots release at the
    subtract (squares write a separate scratch), keeping triggers a
    full square-latency ahead.
  - the last 4096 columns of row block 3 taper 2048/1024/512/512 and
    stream LAST, in dedicated one-shot buffers (no pool waits): compute
    (sub ~2.3us + ACT square in parallel) is 2x faster than the
    ~9.6us/pair stream rate, so after the final 512-wide packet only
    ~1.5us of work remains.
  - the tiny ints/wts loads and the pw gathers use the GPSIMD software
    DGE: a [128, small] load is 128 separate packets that round-robin
    1:1 with streaming packets per engine, so on a HW ring ahead of the
    stream they would stall that ring ~(128/16)*packet_dur.
  - subtract on DVE, square+row-sum accumulate on ACT (concurrent
    engines), one [128, 3] output store at the end.

The device emits per-partition partial sums [128, 3]; the host sums the
8 cores' partials in f64 and forms the three means.
"""

import numpy as np

import concourse.bacc as bacc
import concourse.bass as bass
import concourse.mybir as mybir
from concourse import tile
from concourse.bass_utils import run_bass_kernel_spmd

F32 = mybir.dt.float32
I32 = mybir.dt.int32
OP = mybir.AluOpType

B = 4096
N = 8192
NCORES = 8
BL = B // NCORES          # rows per core = 512
P = 128                   # SBUF partitions
RB = BL // P              # row groups per partition = 4
W = 3                     # gather window width
FT = 4096                 # streaming tile width

# (row_block, col_start, width) in STREAM order.  The taper sits at the
# end of the STREAM order (cross-block): compute is 2x faster than the
# stream, so everything overlaps except the last-landing tiles.
TILES = [
    (3, 0, 4096),
    (0, 0, 4096), (0, 4096, 4096),
    (1, 0, 4096), (1, 4096, 4096),
    (2, 0, 4096), (2, 4096, 4096),
    (3, 4096, 2048),
    (3, 6144, 1024),
    (3, 7168, 512), (3, 7680, 512),
]
NT = len(TILES)           # 11: 7 big + 4 taper
NFILL = 3


def build_nc(debug=False):
    # Bacc (not plain Bass): its compile pipeline runs
    # generate_event_semaphores, which splits multi-sem waits into separate
    # event instructions — TRN2 allows at most 1 embedded wait per
    # instruction, and walrus codegen rejects the unsplit form.
    nc = bacc.Bacc()

    pred = nc.dram_tensor("pred", [BL, N], F32, kind="ExternalInput")
    targ = nc.dram_tensor("targ", [BL, N], F32, kind="ExternalInput")
    # host-computed: flat gather offsets (row*N + clip(idx-1, 0, N-3)),
    # row r = p*RB + q
    ints = nc.dram_tensor("ints", [P, RB], I32, kind="ExternalInput")
    # host-computed weights: [:, 0:12] = f(c) one-hot select,
    # [:, 12:24] = f'(c) (+1/-1)/denom finite-difference weights,
    # both laid out [128, RB*W]
    wts = nc.dram_tensor("wts", [P, 2 * RB * W], F32, kind="ExternalInput")
    partials = nc.dram_tensor("partials", [P, 3], F32, kind="ExternalOutput")
    if debug:
        dbg = nc.dram_tensor("dbg", [P, 24], F32, kind="ExternalOutput")

    def view3(t):  # [128, 12] AP -> [128, 4, 3] AP
        return t.rearrange("p (q k) -> p q k", k=W)

    with tile.TileContext(nc) as tc:
        with (
            tc.tile_pool(name="ppool", bufs=3) as ppool,
            tc.tile_pool(name="tpool", bufs=3) as tpool,
            tc.tile_pool(name="dpool", bufs=2) as dpool,
            tc.tile_pool(name="pb", bufs=1) as pb,
        ):
            # tiny loads via SWDGE so neither HW ring stalls on them
            ints_t = pb.tile([P, RB], I32)
            nc.gpsimd.dma_start(ints_t[:], ints[:, :])
            wts_t = pb.tile([P, 2 * RB * W], F32)
            nc.gpsimd.dma_start(wts_t[:], wts[:, :])

            parts = pb.tile([P, NT], F32)
            po = pb.tile([P, 3], F32)

            pt = [None] * NT
            tt = [None] * NT

            def load(k):
                rb, cs, w = TILES[k]
                rs = rb * P
                if w == FT:
                    pt[k] = ppool.tile([P, FT], F32, name="pt")
                    tt[k] = tpool.tile([P, FT], F32, name="tt")
                else:
                    pt[k] = pb.tile([P, w], F32, name=f"ptl{k}")
                    tt[k] = pb.tile([P, w], F32, name=f"ttl{k}")
                nc.sync.dma_start(pt[k][:, :w], pred[rs:rs + P, cs:cs + w])
                nc.scalar.dma_start(tt[k][:, :w], targ[rs:rs + P, cs:cs + w])

            def compute(k):
                _, _, w = TILES[k]
                dt = dpool.tile([P, FT], F32, name="dt")
                nc.vector.tensor_tensor(out=dt[:, :w], in0=pt[k][:, :w],
                                        in1=tt[k][:, :w], op=OP.subtract)
                # ACT square runs concurrently with the next DVE subtract;
                # in place over dt (values dead, only accum matters); dt has
                # 2 rotating slots so sub_{k+1} waits sq_{k-1}, not sq_k
                nc.scalar.activation(
                    out=dt[:, :w], in_=dt[:, :w],
                    func=mybir.ActivationFunctionType.Square,
                    accum_out=parts[:, k:k + 1],
                )

            for k in range(NFILL):
                load(k)

            # gathers: 3-wide pred window per row via SWDGE; one offset
            # per partition per instruction (HW honors only one)
            pw = pb.tile([P, RB * W], F32)
            for q in range(RB):
                nc.gpsimd.indirect_dma_start(
                    out=pw[:, W * q:W * q + W], out_offset=None,
                    in_=pred[:, :],
                    in_offset=bass.IndirectOffsetOnAxis(
                        ap=ints_t[:, q:q + 1], axis=1),
                )

            for k in range(NT):
                compute(k)
                if k == 0:
                    # f(c) / f'(c): weighted 3-window sums; off the
                    # streaming critical path
                    sel = pb.tile([P, RB * W], F32)
                    nc.vector.tensor_tensor(out=sel[:], in0=wts_t[:, :RB * W],
                                            in1=pw[:], op=OP.mult)
                    fpc = pb.tile([P, RB], F32)
                    nc.vector.reduce_sum(out=fpc[:], in_=view3(sel[:]),
                                         axis=mybir.AxisListType.X)
                    fdw = pb.tile([P, RB * W], F32)
                    nc.vector.tensor_tensor(out=fdw[:], in0=wts_t[:, RB * W:],
                                            in1=pw[:], op=OP.mult)
                    fpp = pb.tile([P, RB], F32)
                    nc.vector.reduce_sum(out=fpp[:], in_=view3(fdw[:]),
                                         axis=mybir.AxisListType.X)
                    # term2: (f(c) - 1)^2; term3: f'(c)^2 — on DVE so the
                    # Scalar stream stays triggers + stream squares only
                    fpm1 = pb.tile([P, RB], F32)
                    nc.vector.tensor_scalar(out=fpm1[:], in0=fpc[:],
                                            scalar1=-1.0, scalar2=None,
                                            op0=OP.add)
                    sq2 = pb.tile([P, RB], F32)
                    nc.vector.scalar_tensor_tensor(
                        out=sq2[:], in0=fpm1[:], scalar=1.0, in1=fpm1[:],
                        op0=OP.mult, op1=OP.mult, accum_out=po[:, 1:2])
                    sq3 = pb.tile([P, RB], F32)
                    nc.vector.scalar_tensor_tensor(
                        out=sq3[:], in0=fpp[:], scalar=1.0, in1=fpp[:],
                        op0=OP.mult, op1=OP.mult, accum_out=po[:, 2:3])
                    if debug:
                        dbt = pb.tile([P, 24], F32)
                        nc.vector.tensor_copy(out=dbt[:, 0:12], in_=pw[:])
                        nc.vector.tensor_copy(out=dbt[:, 12:16], in_=fpc[:])
                        nc.vector.tensor_copy(out=dbt[:, 16:20], in_=fpp[:])
                        offf = pb.tile([P, RB], F32)
                        nc.vector.tensor_copy(out=offf[:], in_=ints_t[:])
                        nc.vector.tensor_copy(out=dbt[:, 20:24], in_=offf[:])
                        nc.sync.dma_start(dbg[:, :], dbt[:])
                if k + NFILL < NT:
                    load(k + NFILL)

            nc.vector.reduce_sum(out=po[:, 0:1], in_=parts[:],
                                 axis=mybir.AxisListType.X)
            nc.sync.dma_start(partials[:, :], po[:])

    return nc


_NC_CACHE = None


def _get_nc():
    global _NC_CACHE
    if _NC_CACHE is None:
        nc = build_nc()
        # Bacc runs its compile pipeline (register alloc, sync-wait
        # splitting) in finalize; the PJRT exec path requires it.
        nc.finalize()
        _NC_CACHE = nc
    return _NC_CACHE


def _host_index_prep(c, x):
    """Exact replication of the reference index math on the tiny inputs.

    idx = argmin_j |x_j - c_r| with numpy f32 ops — bit-identical values
    and the same first-index tie-break as jnp.argmin on CPU.
    Returns flat gather offsets into each core's [BL, N] pred slice and
    the f(c)/f'(c) window weights.
    """
    Bfull = c.shape[0]
    idx = np.empty(Bfull, dtype=np.int64)
    CH = 512
    for s in range(0, Bfull, CH):
        e = min(s + CH, Bfull)
        d = np.abs(x[None, :] - c[s:e, None])  # f32
        idx[s:e] = np.argmin(d, axis=1)
    dx = np.float32(x[1]) - np.float32(x[0])

    ip = np.minimum(idx + 1, N - 1)
    im = np.maximum(idx - 1, 0)
    s3 = np.clip(idx - 1, 0, N - W)           # window start
    p0 = (idx - s3).astype(np.int64)          # positions in window
    pm = (im - s3).astype(np.int64)
    pp = (ip - s3).astype(np.int64)
    denom = (ip - im).astype(np.float32) * dx
    rden = np.float32(1.0) / denom

    rows = np.arange(Bfull)
    wfpc = np.zeros((Bfull, W), dtype=np.float32)
    wfpc[rows, p0] = 1.0
    wfpp = np.zeros((Bfull, W), dtype=np.float32)
    # += not =: pm and pp never collide (ip > im always), but keep the
    # accumulate form cheap and safe
    np.add.at(wfpp, (rows, pp), rden)
    np.add.at(wfpp, (rows, pm), -rden)

    row_in_core = np.arange(Bfull) % BL
    offs = (row_in_core * N + s3).astype(np.int32)
    return offs, wfpc, wfpp


def make_in_maps(predicted_solution_batch, target_solution_batch,
                 c_input_batch, x_eval_points):
    pred = np.ascontiguousarray(predicted_solution_batch, dtype=np.float32)
    targ = np.ascontiguousarray(target_solution_batch, dtype=np.float32)
    c = np.ascontiguousarray(c_input_batch, dtype=np.float32)
    x = np.ascontiguousarray(x_eval_points, dtype=np.float32)
    offs, wfpc, wfpp = _host_index_prep(c, x)

    in_maps = []
    for i in range(NCORES):
        sl = slice(i * BL, (i + 1) * BL)
        # row r in core = p*RB + q  ->  [P, RB] / [P, RB*W] layouts
        wf1 = wfpc[sl].reshape(P, RB * W)
        wf2 = wfpp[sl].reshape(P, RB * W)
        in_maps.append({
            "pred": pred[sl],
            "targ": targ[sl],
            "ints": offs[sl].reshape(P, RB),
            "wts": np.ascontiguousarray(np.concatenate([wf1, wf2], axis=1)),
        })
    return in_maps


def reduce_partials(results):
    s = np.zeros(3, dtype=np.float64)
    for r in results:
        s += r["partials"].astype(np.float64).sum(axis=0)
    loss = s[0] / (B * N) + s[1] / B + s[2] / B
    return np.float32(loss)


def kernel(predicted_solution_batch, target_solution_batch,
           c_input_batch, x_eval_points):
    nc = _get_nc()
    in_maps = make_in_maps(predicted_solution_batch, target_solution_batch,
                           c_input_batch, x_eval_points)
    res = run_bass_kernel_spmd(nc, in_maps, core_ids=list(range(NCORES)))
    return reduce_partials(res.results)
